# revision 6
# baseline (speedup 1.0000x reference)
"""GNN aggregator (KGAT-style bi-interaction) Trainium2 kernel.

side = segment_sum(edge_val * ego[edge_col], edge_row)       # SpMM, COO
out  = LN(leaky_relu((ego+side)@W1+b1)) + LN(leaky_relu((ego*side)@W2+b2))

Sharding: edges partitioned by destination row across 8 cores; core c owns
rows [c*12500, (c+1)*12500) and computes its slice of the output. The gather
side reads a replicated fp16 copy of ego straight out of HBM with dma_gather
(int16 indices -> 4 chunks of 25000 rows). Segment-sum is done on the tensor
engine: for each 128-edge block, a [128e x 128dest] scatter matrix S
(S[e,d] = val_e * (d == destloc_e), built by one dual-op tensor_scalar on DVE)
is multiplied against the gathered messages, accumulating side^T[d, dest] in
PSUM. The per-node dense chain runs in fp16 with fp32 accumulation.
"""

import math

import numpy as np

import concourse.bacc as bacc
import concourse.bass as bass
import concourse.mybir as mybir
import concourse.tile as tile
from concourse import library_config
from concourse.bass_utils import run_bass_kernel_spmd

F16 = mybir.dt.float16
F32 = mybir.dt.float32
I16 = mybir.dt.int16
ALU = mybir.AluOpType
ACTF = mybir.ActivationFunctionType

N = 100000
D = 128
NCORES = 8
ROWS_PER_CORE = N // NCORES          # 12500
NTILES = math.ceil(ROWS_PER_CORE / 128)   # 98 (last tile 84 rows)
TILES_PER_ST = 7
NST = NTILES // TILES_PER_ST         # 14
CHUNK_ROWS = 25000                   # <= 32767 for int16 indices
NCHUNKS = math.ceil(N / CHUNK_ROWS)  # 4
LN_EPS = 1e-5
NEG_SLOPE = 0.01
# fraction of S-build tensor_scalar ops routed to gpsimd to offload DVE
GPSIMD_S_EVERY = 0        # 0 = disabled; k>0 -> every k-th block on gpsimd
DMA_SCRATCH = 16384      # descriptor-ring carveout bytes (default 16384 = 1024 descs)


def _tile_width(t):
    return min(128, ROWS_PER_CORE - t * 128)


def _preprocess(ego, edge_row, edge_col, edge_val):
    """Sort/shard edges on the host. Returns (B, per-core input maps' parts).

    B[t][c]: static number of 128-edge blocks for (tile, chunk) -- max over
    cores, shared by the SPMD program.
    """
    edge_row = np.asarray(edge_row).astype(np.int64)
    edge_col = np.asarray(edge_col).astype(np.int64)
    edge_val = np.asarray(edge_val).astype(np.float32)

    core = edge_row // ROWS_PER_CORE
    local = edge_row - core * ROWS_PER_CORE
    tl = local // 128
    destloc = local - tl * 128
    ch = edge_col // CHUNK_ROWS
    idx16 = (edge_col - ch * CHUNK_ROWS).astype(np.int16)

    key = ((core * NTILES + tl) * NCHUNKS + ch).astype(np.int64)
    order = np.argsort(key, kind="stable")
    key_s = key[order]
    idx16_s = idx16[order]
    destloc_s = destloc[order].astype(np.float32)
    val_s = edge_val[order]

    counts = np.bincount(key_s, minlength=NCORES * NTILES * NCHUNKS).reshape(
        NCORES, NTILES, NCHUNKS
    )
    group_start = np.zeros(NCORES * NTILES * NCHUNKS + 1, np.int64)
    np.cumsum(counts.reshape(-1), out=group_start[1:])

    B = np.ceil(counts / 128).astype(np.int64).max(axis=0)  # [NTILES, NCHUNKS]
    # guarantee at least one block per tile so every PSUM group is written
    empty = B.sum(axis=1) == 0
    B[empty, 0] = 1

    B_t = B.sum(axis=1)                       # blocks per tile
    TOTB = int(B_t.sum())

    # per-(st, c) call sizes and offsets
    NI = np.zeros((NST, NCHUNKS), np.int64)   # num idxs per gather call
    for st in range(NST):
        for c in range(NCHUNKS):
            NI[st, c] = 128 * int(B[st * TILES_PER_ST : (st + 1) * TILES_PER_ST, c].sum())
    idx_cols_per_call = NI // 16
    idx_col_off = np.zeros((NST, NCHUNKS), np.int64)
    acc = 0
    for st in range(NST):
        for c in range(NCHUNKS):
            idx_col_off[st, c] = acc
            acc += idx_cols_per_call[st, c]
    IDXCOLS = int(acc)

    # block offset of tile t inside its (st, c) gather call
    call_blk_off = np.zeros((NTILES, NCHUNKS), np.int64)
    for st in range(NST):
        for c in range(NCHUNKS):
            off = 0
            for t in range(st * TILES_PER_ST, (st + 1) * TILES_PER_ST):
                call_blk_off[t, c] = off
                off += B[t, c]

    # column base of tile t in the dl/val arrays
    tb_base = np.zeros(NTILES + 1, np.int64)
    np.cumsum(B_t, out=tb_base[1:])

    per_core = []
    for corei in range(NCORES):
        idx_all = np.zeros((16, IDXCOLS), np.int16)
        dl_all = np.zeros((128, TOTB), np.float32)
        val_all = np.zeros((128, TOTB), np.float32)
        for t in range(NTILES):
            st = t // TILES_PER_ST
            jc = 0  # block index within the tile (chunk-major)
            for c in range(NCHUNKS):
                g = (corei * NTILES + t) * NCHUNKS + c
                s0, s1 = group_start[g], group_start[g + 1]
                n = int(s1 - s0)
                nb = int(B[t, c])
                if nb == 0:
                    continue
                # destination slots inside this (st,c) call
                base_slot = call_blk_off[t, c] * 128
                sl = np.arange(n) + base_slot
                col0 = idx_col_off[st, c]
                idx_all[sl % 16, col0 + sl // 16] = idx16_s[s0:s1]
                # dl/val columns for this tile's blocks
                jcols = tb_base[t] + jc + np.arange(n) // 128
                lanes = np.arange(n) % 128
                dl_all[lanes, jcols] = destloc_s[s0:s1]
                val_all[lanes, jcols] = val_s[s0:s1]
                jc += nb
        per_core.append(
            {
                "idx": np.tile(idx_all, (8, 1)),
                "dl": dl_all,
                "val": val_all,
            }
        )

    meta = dict(
        B=B,
        B_t=B_t,
        TOTB=TOTB,
        NI=NI,
        idx_cols_per_call=idx_cols_per_call,
        idx_col_off=idx_col_off,
        call_blk_off=call_blk_off,
        tb_base=tb_base,
        IDXCOLS=IDXCOLS,
    )
    return meta, per_core


def _build_program(meta, affine):
    B = meta["B"]
    B_t = meta["B_t"]
    TOTB = meta["TOTB"]
    NI = meta["NI"]
    idx_cols_per_call = meta["idx_cols_per_call"]
    idx_col_off = meta["idx_col_off"]
    call_blk_off = meta["call_blk_off"]
    tb_base = meta["tb_base"]
    IDXCOLS = meta["IDXCOLS"]

    nc = bacc.Bacc("TRN2", target_bir_lowering=False, debug=False, num_devices=NCORES,
                   dynamic_dma_scratch_size=DMA_SCRATCH)

    table = nc.dram_tensor("table", [N, D], F16, kind="ExternalInput")
    egoT = nc.dram_tensor("egoT", [D, ROWS_PER_CORE], F16, kind="ExternalInput")
    idx_d = nc.dram_tensor("idx", [128, IDXCOLS], I16, kind="ExternalInput")
    dl_d = nc.dram_tensor("dl", [128, TOTB], F32, kind="ExternalInput")
    val_d = nc.dram_tensor("val", [128, TOTB], F32, kind="ExternalInput")
    W1_d = nc.dram_tensor("W1", [D, D], F16, kind="ExternalInput")
    W2_d = nc.dram_tensor("W2", [D, D], F16, kind="ExternalInput")
    b1_d = nc.dram_tensor("b1", [1, D], F16, kind="ExternalInput")
    b2_d = nc.dram_tensor("b2", [1, D], F16, kind="ExternalInput")
    if affine:
        g1_d = nc.dram_tensor("g1bc", [128, D], F32, kind="ExternalInput")
        g2_d = nc.dram_tensor("g2bc", [128, D], F32, kind="ExternalInput")
        bs_d = nc.dram_tensor("bsbc", [128, D], F32, kind="ExternalInput")
    out_d = nc.dram_tensor("out", [ROWS_PER_CORE, D], F32, kind="ExternalOutput")

    max_B_st_c = 0
    for st in range(NST):
        for c in range(NCHUNKS):
            max_B_st_c = max(max_B_st_c, int(NI[st, c]) // 128)
    max_B_t = int(B_t.max())
    max_TB_st = 0
    for st in range(NST):
        t0, t1 = st * TILES_PER_ST, (st + 1) * TILES_PER_ST
        max_TB_st = max(max_TB_st, int(tb_base[t1] - tb_base[t0]))
    max_idx_cols = int(idx_cols_per_call.max())

    with tile.TileContext(nc) as tc:
        nc.gpsimd.load_library(library_config.mlp)
        with (
            tc.tile_pool(name="const", bufs=1) as pconst,
            tc.tile_pool(name="idx", bufs=2) as pidx,
            tc.tile_pool(name="msgs", bufs=2) as pmsgs,
            tc.tile_pool(name="dlv", bufs=2) as pdlv,
            tc.tile_pool(name="S", bufs=3) as pS,
            tc.tile_pool(name="ego", bufs=3) as pego,
            tc.tile_pool(name="work", bufs=3) as pwork,
            tc.tile_pool(name="stats", bufs=8) as pstats,
            tc.tile_pool(name="psum", bufs=2, space="PSUM") as ppsum,
        ):
            iota_i = pconst.tile([128, 128], I16)
            nc.gpsimd.iota(iota_i[:], pattern=[[1, 128]], base=0, channel_multiplier=0)
            iota_f = pconst.tile([128, 128], F16)
            nc.vector.tensor_copy(iota_f[:], iota_i[:])

            W1 = pconst.tile([D, D], F16, tag="w1")
            nc.sync.dma_start(W1[:], W1_d[:])
            W2 = pconst.tile([D, D], F16, tag="w2")
            nc.sync.dma_start(W2[:], W2_d[:])
            b1 = pconst.tile([1, D], F16, tag="b1")
            nc.sync.dma_start(b1[:], b1_d[:])
            b2 = pconst.tile([1, D], F16, tag="b2")
            nc.sync.dma_start(b2[:], b2_d[:])
            ones = pconst.tile([1, 128], F16, tag="ones")
            nc.vector.memset(ones[:], 1.0)
            eps_t = pconst.tile([128, 1], F32, tag="eps")
            nc.vector.memset(eps_t[:], LN_EPS)
            if affine:
                g1bc = pconst.tile([128, D], F32, tag="g1")
                nc.sync.dma_start(g1bc[:], g1_d[:])
                g2bc = pconst.tile([128, D], F32, tag="g2")
                nc.sync.dma_start(g2bc[:], g2_d[:])
                bsbc = pconst.tile([128, D], F32, tag="bs")
                nc.sync.dma_start(bsbc[:], bs_d[:])

            for st in range(NST):
                t0, t1 = st * TILES_PER_ST, (st + 1) * TILES_PER_ST
                msgs = [None] * NCHUNKS
                for c in range(NCHUNKS):
                    ni = int(NI[st, c])
                    if ni == 0:
                        continue
                    cols = ni // 16
                    it = pidx.tile([128, max_idx_cols], I16, tag=f"idx{c}")
                    co = int(idx_col_off[st, c])
                    nc.sync.dma_start(it[:, :cols], idx_d[:, co : co + cols])
                    m = pmsgs.tile([128, max_B_st_c, D], F16, tag=f"msgs{c}")
                    nc.gpsimd.dma_gather(
                        out_ap=m[:, : ni // 128, :],
                        in_ap=table[c * CHUNK_ROWS : (c + 1) * CHUNK_ROWS, :],
                        idxs_ap=it[:, :cols],
                        num_idxs=ni,
                        num_idxs_reg=ni,
                        elem_size=D,
                        single_packet=False,
                    )
                    msgs[c] = m

                tb0 = int(tb_base[t0])
                tb_st = int(tb_base[t1]) - tb0
                dl_t = pdlv.tile([128, max_TB_st], F32, tag="dl")
                nc.sync.dma_start(dl_t[:, :tb_st], dl_d[:, tb0 : tb0 + tb_st])
                val_t = pdlv.tile([128, max_TB_st], F32, tag="val")
                nc.sync.dma_start(val_t[:, :tb_st], val_d[:, tb0 : tb0 + tb_st])

                for t in range(t0, t1):
                    w = _tile_width(t)
                    bt = int(B_t[t])
                    St = pS.tile([128, max_B_t, 128], F16, tag="S")
                    ps = ppsum.tile([128, 128], F32, tag="side")
                    j = 0
                    for c in range(NCHUNKS):
                        moff = int(call_blk_off[t, c])
                        for b in range(int(B[t, c])):
                            col = tb_base[t] - tb0 + j
                            eng = nc.vector
                            if GPSIMD_S_EVERY and (j % GPSIMD_S_EVERY) == (
                                GPSIMD_S_EVERY - 1
                            ):
                                eng = nc.gpsimd
                            eng.tensor_scalar(
                                out=St[:, j, :w],
                                in0=iota_f[:, :w],
                                scalar1=dl_t[:, col : col + 1],
                                scalar2=val_t[:, col : col + 1],
                                op0=ALU.is_equal,
                                op1=ALU.mult,
                            )
                            nc.tensor.matmul(
                                ps[:, :w],
                                msgs[c][:, moff + b, :],
                                St[:, j, :w],
                                start=(j == 0),
                                stop=(j == bt - 1),
                            )
                            j += 1

                    ego_t = pego.tile([128, 128], F16, tag="egoT")
                    nc.sync.dma_start(ego_t[:, :w], egoT[:, t * 128 : t * 128 + w])

                    side_sb = pwork.tile([128, 128], F16, tag="side_sb")
                    nc.scalar.activation(side_sb[:, :w], ps[:, :w], ACTF.Copy)

                    x1 = pwork.tile([128, 128], F16, tag="x1")
                    nc.vector.tensor_tensor(x1[:, :w], ego_t[:, :w], side_sb[:, :w], ALU.add)
                    x2 = pwork.tile([128, 128], F16, tag="x2")
                    nc.vector.tensor_tensor(x2[:, :w], ego_t[:, :w], side_sb[:, :w], ALU.mult)

                    ys = []
                    for bi, (x, Wt, bt_) in enumerate(((x1, W1, b1), (x2, W2, b2))):
                        po = ppsum.tile([128, 128], F32, tag=f"o{bi}")
                        nc.tensor.matmul(po[:w, :], x[:, :w], Wt[:], start=True, stop=False)
                        nc.tensor.matmul(
                            po[:w, :], ones[:1, :w], bt_[:1, :], start=False, stop=True
                        )
                        h_sb = pwork.tile([128, 128], F16, tag=f"h{bi}")
                        nc.scalar.activation(h_sb[:w, :], po[:w, :], ACTF.Copy)
                        t_act = pwork.tile([128, 128], F16, tag=f"t{bi}")
                        s1 = pstats.tile([128, 1], F32, tag=f"sum{bi}")
                        nc.vector.scalar_tensor_tensor(
                            out=t_act[:w, :],
                            in0=h_sb[:w, :],
                            scalar=NEG_SLOPE,
                            in1=h_sb[:w, :],
                            op0=ALU.mult,
                            op1=ALU.max,
                            accum_out=s1[:w, :],
                        )
                        sq = pwork.tile([128, 128], F16, tag=f"sq{bi}")
                        ssq = pstats.tile([128, 1], F32, tag=f"ssq{bi}")
                        nc.scalar.activation(
                            sq[:w, :], t_act[:w, :], ACTF.Square, accum_out=ssq[:w, :]
                        )
                        mu = pstats.tile([128, 1], F32, tag=f"mu{bi}")
                        nc.vector.tensor_scalar(
                            out=mu[:w, :], in0=s1[:w, :], scalar1=1.0 / D,
                            scalar2=None, op0=ALU.mult,
                        )
                        mu2 = pstats.tile([128, 1], F32, tag=f"mu2{bi}")
                        nc.vector.tensor_scalar(
                            out=mu2[:w, :], in0=mu[:w, :], scalar1=mu[:w, :],
                            scalar2=None, op0=ALU.mult,
                        )
                        var = pstats.tile([128, 1], F32, tag=f"var{bi}")
                        nc.vector.tensor_scalar(
                            out=var[:w, :], in0=ssq[:w, :], scalar1=1.0 / D,
                            scalar2=mu2[:w, :], op0=ALU.mult, op1=ALU.subtract,
                        )
                        std = pstats.tile([128, 1], F32, tag=f"std{bi}")
                        nc.scalar.activation(
                            std[:w, :], var[:w, :], ACTF.Sqrt, bias=eps_t[:w, :]
                        )
                        rstd = pstats.tile([128, 1], F32, tag=f"rstd{bi}")
                        nc.vector.reciprocal(rstd[:w, :], std[:w, :])
                        y = pwork.tile([128, 128], F16, tag=f"y{bi}")
                        nc.vector.tensor_scalar(
                            out=y[:w, :], in0=t_act[:w, :], scalar1=mu[:w, :],
                            scalar2=rstd[:w, :], op0=ALU.subtract, op1=ALU.mult,
                        )
                        ys.append(y)

                    out_t = pwork.tile([128, 128], F32, tag="out")
                    if affine:
                        a1 = pwork.tile([128, 128], F32, tag="a1")
                        nc.vector.tensor_tensor(a1[:w, :], ys[0][:w, :], g1bc[:w, :], ALU.mult)
                        a2 = pwork.tile([128, 128], F32, tag="a2")
                        nc.vector.tensor_tensor(a2[:w, :], ys[1][:w, :], g2bc[:w, :], ALU.mult)
                        nc.vector.tensor_tensor(a1[:w, :], a1[:w, :], a2[:w, :], ALU.add)
                        nc.vector.tensor_tensor(out_t[:w, :], a1[:w, :], bsbc[:w, :], ALU.add)
                    else:
                        nc.vector.tensor_tensor(out_t[:w, :], ys[0][:w, :], ys[1][:w, :], ALU.add)
                    nc.sync.dma_start(out_d[t * 128 : t * 128 + w, :], out_t[:w, :])

    nc.compile()
    return nc


def kernel(
    ego_embeddings,
    edge_row,
    edge_col,
    edge_val,
    W1,
    b1,
    W2,
    b2,
    gamma1,
    beta1,
    gamma2,
    beta2,
):
    ego = np.asarray(ego_embeddings, np.float32)
    W1 = np.asarray(W1, np.float32)
    W2 = np.asarray(W2, np.float32)
    b1 = np.asarray(b1, np.float32)
    b2 = np.asarray(b2, np.float32)
    gamma1 = np.asarray(gamma1, np.float32)
    gamma2 = np.asarray(gamma2, np.float32)
    beta1 = np.asarray(beta1, np.float32)
    beta2 = np.asarray(beta2, np.float32)

    affine = not (
        np.all(gamma1 == 1.0)
        and np.all(gamma2 == 1.0)
        and np.all(beta1 == 0.0)
        and np.all(beta2 == 0.0)
    )

    meta, per_core = _preprocess(ego, edge_row, edge_col, edge_val)
    nc = _build_program(meta, affine)

    table = np.ascontiguousarray(ego.astype(np.float16))
    in_maps = []
    for c in range(NCORES):
        r0 = c * ROWS_PER_CORE
        m = {
            "table": table,
            "egoT": np.ascontiguousarray(ego[r0 : r0 + ROWS_PER_CORE].T.astype(np.float16)),
            "idx": per_core[c]["idx"],
            "dl": per_core[c]["dl"],
            "val": per_core[c]["val"],
            "W1": W1.astype(np.float16),
            "W2": W2.astype(np.float16),
            "b1": b1.astype(np.float16).reshape(1, D),
            "b2": b2.astype(np.float16).reshape(1, D),
        }
        if affine:
            m["g1bc"] = np.tile(gamma1.reshape(1, D), (128, 1)).astype(np.float32)
            m["g2bc"] = np.tile(gamma2.reshape(1, D), (128, 1)).astype(np.float32)
            m["bsbc"] = np.tile((beta1 + beta2).reshape(1, D), (128, 1)).astype(np.float32)
        in_maps.append(m)

    res = run_bass_kernel_spmd(nc, in_maps, core_ids=list(range(NCORES)))
    return np.concatenate([res.results[c]["out"] for c in range(NCORES)], axis=0)


# revision 8
# speedup vs baseline: 1.7446x; 1.7446x over previous
"""GNN aggregator (KGAT-style bi-interaction) Trainium2 kernel.

side = segment_sum(edge_val * ego[edge_col], edge_row)       # SpMM, COO
out  = LN(leaky_relu((ego+side)@W1+b1)) + LN(leaky_relu((ego*side)@W2+b2))

Sharding: edges partitioned by destination row across 8 cores; core c owns
rows [c*12500, (c+1)*12500) and computes its slice of the output. The gather
side reads a replicated fp16 copy of ego straight out of HBM with dma_gather
(int16 indices -> 4 chunks of 25000 rows). Segment-sum is done on the tensor
engine: for each 128-edge block, a [128e x 128dest] scatter matrix S
(S[e,d] = val_e * (d == destloc_e), built by one dual-op tensor_scalar on DVE)
is multiplied against the gathered messages, accumulating side^T[d, dest] in
PSUM. The per-node dense chain runs in fp16 with fp32 accumulation.
"""

import math

import numpy as np

import concourse.bacc as bacc
import concourse.bass as bass
import concourse.mybir as mybir
import concourse.tile as tile
from concourse import library_config
from concourse.bass_utils import run_bass_kernel_spmd

F16 = mybir.dt.float16
F32 = mybir.dt.float32
I16 = mybir.dt.int16
ALU = mybir.AluOpType
ACTF = mybir.ActivationFunctionType

N = 100000
D = 128
NCORES = 8
ROWS_PER_CORE = N // NCORES          # 12500
NTILES = math.ceil(ROWS_PER_CORE / 128)   # 98 (last tile 84 rows)
TILES_PER_ST = 7
NST = NTILES // TILES_PER_ST         # 14
CHUNK_ROWS = 25000                   # <= 32767 for int16 indices
NCHUNKS = math.ceil(N / CHUNK_ROWS)  # 4
LN_EPS = 1e-5
NEG_SLOPE = 0.01
# fraction of S-build tensor_scalar ops routed to gpsimd to offload DVE
GPSIMD_S_EVERY = 0        # 0 = disabled; k>0 -> every k-th block on gpsimd
DMA_SCRATCH = 16384
SINGLE_PACKET = False
NUM_SWDGE_QUEUES = 4      # descriptor-ring carveout bytes (default 16384 = 1024 descs)


def _tile_width(t):
    return min(128, ROWS_PER_CORE - t * 128)


def _preprocess(ego, edge_row, edge_col, edge_val):
    """Sort/shard edges on the host. Returns (B, per-core input maps' parts).

    B[t][c]: static number of 128-edge blocks for (tile, chunk) -- max over
    cores, shared by the SPMD program.
    """
    edge_row = np.asarray(edge_row).astype(np.int64)
    edge_col = np.asarray(edge_col).astype(np.int64)
    edge_val = np.asarray(edge_val).astype(np.float32)

    core = edge_row // ROWS_PER_CORE
    local = edge_row - core * ROWS_PER_CORE
    tl = local // 128
    destloc = local - tl * 128
    ch = edge_col // CHUNK_ROWS
    idx16 = (edge_col - ch * CHUNK_ROWS).astype(np.int16)

    key = ((core * NTILES + tl) * NCHUNKS + ch).astype(np.int64)
    order = np.argsort(key, kind="stable")
    key_s = key[order]
    idx16_s = idx16[order]
    destloc_s = destloc[order].astype(np.float32)
    val_s = edge_val[order]

    counts = np.bincount(key_s, minlength=NCORES * NTILES * NCHUNKS).reshape(
        NCORES, NTILES, NCHUNKS
    )
    group_start = np.zeros(NCORES * NTILES * NCHUNKS + 1, np.int64)
    np.cumsum(counts.reshape(-1), out=group_start[1:])

    B = np.ceil(counts / 128).astype(np.int64).max(axis=0)  # [NTILES, NCHUNKS]
    # guarantee at least one block per tile so every PSUM group is written
    empty = B.sum(axis=1) == 0
    B[empty, 0] = 1

    B_t = B.sum(axis=1)                       # blocks per tile
    TOTB = int(B_t.sum())

    # per-(st, c) call sizes and offsets
    NI = np.zeros((NST, NCHUNKS), np.int64)   # num idxs per gather call
    for st in range(NST):
        for c in range(NCHUNKS):
            NI[st, c] = 128 * int(B[st * TILES_PER_ST : (st + 1) * TILES_PER_ST, c].sum())
    idx_cols_per_call = NI // 16
    idx_col_off = np.zeros((NST, NCHUNKS), np.int64)
    acc = 0
    for st in range(NST):
        for c in range(NCHUNKS):
            idx_col_off[st, c] = acc
            acc += idx_cols_per_call[st, c]
    IDXCOLS = int(acc)

    # block offset of tile t inside its (st, c) gather call
    call_blk_off = np.zeros((NTILES, NCHUNKS), np.int64)
    for st in range(NST):
        for c in range(NCHUNKS):
            off = 0
            for t in range(st * TILES_PER_ST, (st + 1) * TILES_PER_ST):
                call_blk_off[t, c] = off
                off += B[t, c]

    # column base of tile t in the dl/val arrays
    tb_base = np.zeros(NTILES + 1, np.int64)
    np.cumsum(B_t, out=tb_base[1:])

    per_core = []
    for corei in range(NCORES):
        idx_all = np.zeros((16, IDXCOLS), np.int16)
        dl_all = np.zeros((128, TOTB), np.float32)
        val_all = np.zeros((128, TOTB), np.float32)
        for t in range(NTILES):
            st = t // TILES_PER_ST
            jc = 0  # block index within the tile (chunk-major)
            for c in range(NCHUNKS):
                g = (corei * NTILES + t) * NCHUNKS + c
                s0, s1 = group_start[g], group_start[g + 1]
                n = int(s1 - s0)
                nb = int(B[t, c])
                if nb == 0:
                    continue
                # destination slots inside this (st,c) call
                base_slot = call_blk_off[t, c] * 128
                sl = np.arange(n) + base_slot
                col0 = idx_col_off[st, c]
                idx_all[sl % 16, col0 + sl // 16] = idx16_s[s0:s1]
                # dl/val columns for this tile's blocks
                jcols = tb_base[t] + jc + np.arange(n) // 128
                lanes = np.arange(n) % 128
                dl_all[lanes, jcols] = destloc_s[s0:s1]
                val_all[lanes, jcols] = val_s[s0:s1]
                jc += nb
        per_core.append(
            {
                "idx": np.tile(idx_all, (8, 1)),
                "dl": dl_all,
                "val": val_all,
            }
        )

    meta = dict(
        B=B,
        B_t=B_t,
        TOTB=TOTB,
        NI=NI,
        idx_cols_per_call=idx_cols_per_call,
        idx_col_off=idx_col_off,
        call_blk_off=call_blk_off,
        tb_base=tb_base,
        IDXCOLS=IDXCOLS,
    )
    return meta, per_core


def _build_program(meta, affine):
    B = meta["B"]
    B_t = meta["B_t"]
    TOTB = meta["TOTB"]
    NI = meta["NI"]
    idx_cols_per_call = meta["idx_cols_per_call"]
    idx_col_off = meta["idx_col_off"]
    call_blk_off = meta["call_blk_off"]
    tb_base = meta["tb_base"]
    IDXCOLS = meta["IDXCOLS"]

    nc = bacc.Bacc("TRN2", target_bir_lowering=False, debug=False, num_devices=NCORES,
                   dynamic_dma_scratch_size=DMA_SCRATCH, num_swdge_queues=NUM_SWDGE_QUEUES)

    table = nc.dram_tensor("table", [N, D], F16, kind="ExternalInput")
    egoT = nc.dram_tensor("egoT", [D, ROWS_PER_CORE], F16, kind="ExternalInput")
    idx_d = nc.dram_tensor("idx", [128, IDXCOLS], I16, kind="ExternalInput")
    dl_d = nc.dram_tensor("dl", [128, TOTB], F32, kind="ExternalInput")
    val_d = nc.dram_tensor("val", [128, TOTB], F32, kind="ExternalInput")
    W1_d = nc.dram_tensor("W1", [D, D], F16, kind="ExternalInput")
    W2_d = nc.dram_tensor("W2", [D, D], F16, kind="ExternalInput")
    b1_d = nc.dram_tensor("b1", [1, D], F16, kind="ExternalInput")
    b2_d = nc.dram_tensor("b2", [1, D], F16, kind="ExternalInput")
    if affine:
        g1_d = nc.dram_tensor("g1bc", [128, D], F32, kind="ExternalInput")
        g2_d = nc.dram_tensor("g2bc", [128, D], F32, kind="ExternalInput")
        bs_d = nc.dram_tensor("bsbc", [128, D], F32, kind="ExternalInput")
    out_d = nc.dram_tensor("out", [ROWS_PER_CORE, D], F32, kind="ExternalOutput")

    max_B_st_c = 0
    for st in range(NST):
        for c in range(NCHUNKS):
            max_B_st_c = max(max_B_st_c, int(NI[st, c]) // 128)
    max_B_t = int(B_t.max())
    max_TB_st = 0
    for st in range(NST):
        t0, t1 = st * TILES_PER_ST, (st + 1) * TILES_PER_ST
        max_TB_st = max(max_TB_st, int(tb_base[t1] - tb_base[t0]))
    max_idx_cols = int(idx_cols_per_call.max())

    with tile.TileContext(nc) as tc:
        nc.gpsimd.load_library(library_config.mlp)
        with (
            tc.tile_pool(name="const", bufs=1) as pconst,
            tc.tile_pool(name="idx", bufs=2) as pidx,
            tc.tile_pool(name="msgs", bufs=2) as pmsgs,
            tc.tile_pool(name="dlv", bufs=2) as pdlv,
            tc.tile_pool(name="S", bufs=3) as pS,
            tc.tile_pool(name="ego", bufs=3) as pego,
            tc.tile_pool(name="work", bufs=3) as pwork,
            tc.tile_pool(name="stats", bufs=8) as pstats,
            tc.tile_pool(name="psum", bufs=2, space="PSUM") as ppsum,
        ):
            iota_i = pconst.tile([128, 128], I16)
            nc.gpsimd.iota(iota_i[:], pattern=[[1, 128]], base=0, channel_multiplier=0)
            iota_f = pconst.tile([128, 128], F16)
            nc.vector.tensor_copy(iota_f[:], iota_i[:])

            W1 = pconst.tile([D, D], F16, tag="w1")
            nc.sync.dma_start(W1[:], W1_d[:])
            W2 = pconst.tile([D, D], F16, tag="w2")
            nc.sync.dma_start(W2[:], W2_d[:])
            b1 = pconst.tile([1, D], F16, tag="b1")
            nc.sync.dma_start(b1[:], b1_d[:])
            b2 = pconst.tile([1, D], F16, tag="b2")
            nc.sync.dma_start(b2[:], b2_d[:])
            ones = pconst.tile([1, 128], F16, tag="ones")
            nc.vector.memset(ones[:], 1.0)
            eps_t = pconst.tile([128, 1], F32, tag="eps")
            nc.vector.memset(eps_t[:], LN_EPS)
            if affine:
                g1bc = pconst.tile([128, D], F32, tag="g1")
                nc.sync.dma_start(g1bc[:], g1_d[:])
                g2bc = pconst.tile([128, D], F32, tag="g2")
                nc.sync.dma_start(g2bc[:], g2_d[:])
                bsbc = pconst.tile([128, D], F32, tag="bs")
                nc.sync.dma_start(bsbc[:], bs_d[:])

            for st in range(NST):
                t0, t1 = st * TILES_PER_ST, (st + 1) * TILES_PER_ST
                msgs = [None] * NCHUNKS
                for c in range(NCHUNKS):
                    ni = int(NI[st, c])
                    if ni == 0:
                        continue
                    cols = ni // 16
                    it = pidx.tile([128, max_idx_cols], I16, tag=f"idx{c}")
                    co = int(idx_col_off[st, c])
                    nc.sync.dma_start(it[:, :cols], idx_d[:, co : co + cols])
                    m = pmsgs.tile([128, max_B_st_c, D], F16, tag=f"msgs{c}")
                    nc.gpsimd.dma_gather(
                        out_ap=m[:, : ni // 128, :],
                        in_ap=table[c * CHUNK_ROWS : (c + 1) * CHUNK_ROWS, :],
                        idxs_ap=it[:, :cols],
                        num_idxs=ni,
                        num_idxs_reg=ni,
                        elem_size=D,
                        single_packet=SINGLE_PACKET,
                        queue_num=c % NUM_SWDGE_QUEUES,
                    )
                    msgs[c] = m

                tb0 = int(tb_base[t0])
                tb_st = int(tb_base[t1]) - tb0
                dl_t = pdlv.tile([128, max_TB_st], F32, tag="dl")
                nc.sync.dma_start(dl_t[:, :tb_st], dl_d[:, tb0 : tb0 + tb_st])
                val_t = pdlv.tile([128, max_TB_st], F32, tag="val")
                nc.sync.dma_start(val_t[:, :tb_st], val_d[:, tb0 : tb0 + tb_st])

                for t in range(t0, t1):
                    w = _tile_width(t)
                    bt = int(B_t[t])
                    St = pS.tile([128, max_B_t, 128], F16, tag="S")
                    ps = ppsum.tile([128, 128], F32, tag="side")
                    j = 0
                    for c in range(NCHUNKS):
                        moff = int(call_blk_off[t, c])
                        for b in range(int(B[t, c])):
                            col = tb_base[t] - tb0 + j
                            eng = nc.vector
                            if GPSIMD_S_EVERY and (j % GPSIMD_S_EVERY) == (
                                GPSIMD_S_EVERY - 1
                            ):
                                eng = nc.gpsimd
                            eng.tensor_scalar(
                                out=St[:, j, :w],
                                in0=iota_f[:, :w],
                                scalar1=dl_t[:, col : col + 1],
                                scalar2=val_t[:, col : col + 1],
                                op0=ALU.is_equal,
                                op1=ALU.mult,
                            )
                            nc.tensor.matmul(
                                ps[:, :w],
                                msgs[c][:, moff + b, :],
                                St[:, j, :w],
                                start=(j == 0),
                                stop=(j == bt - 1),
                            )
                            j += 1

                    ego_t = pego.tile([128, 128], F16, tag="egoT")
                    nc.sync.dma_start(ego_t[:, :w], egoT[:, t * 128 : t * 128 + w])

                    side_sb = pwork.tile([128, 128], F16, tag="side_sb")
                    nc.scalar.activation(side_sb[:, :w], ps[:, :w], ACTF.Copy)

                    x1 = pwork.tile([128, 128], F16, tag="x1")
                    nc.vector.tensor_tensor(x1[:, :w], ego_t[:, :w], side_sb[:, :w], ALU.add)
                    x2 = pwork.tile([128, 128], F16, tag="x2")
                    nc.vector.tensor_tensor(x2[:, :w], ego_t[:, :w], side_sb[:, :w], ALU.mult)

                    ys = []
                    for bi, (x, Wt, bt_) in enumerate(((x1, W1, b1), (x2, W2, b2))):
                        po = ppsum.tile([128, 128], F32, tag=f"o{bi}")
                        nc.tensor.matmul(po[:w, :], x[:, :w], Wt[:], start=True, stop=False)
                        nc.tensor.matmul(
                            po[:w, :], ones[:1, :w], bt_[:1, :], start=False, stop=True
                        )
                        h_sb = pwork.tile([128, 128], F16, tag=f"h{bi}")
                        nc.scalar.activation(h_sb[:w, :], po[:w, :], ACTF.Copy)
                        t_act = pwork.tile([128, 128], F16, tag=f"t{bi}")
                        s1 = pstats.tile([128, 1], F32, tag=f"sum{bi}")
                        nc.vector.scalar_tensor_tensor(
                            out=t_act[:w, :],
                            in0=h_sb[:w, :],
                            scalar=NEG_SLOPE,
                            in1=h_sb[:w, :],
                            op0=ALU.mult,
                            op1=ALU.max,
                            accum_out=s1[:w, :],
                        )
                        sq = pwork.tile([128, 128], F16, tag=f"sq{bi}")
                        ssq = pstats.tile([128, 1], F32, tag=f"ssq{bi}")
                        nc.scalar.activation(
                            sq[:w, :], t_act[:w, :], ACTF.Square, accum_out=ssq[:w, :]
                        )
                        mu = pstats.tile([128, 1], F32, tag=f"mu{bi}")
                        nc.vector.tensor_scalar(
                            out=mu[:w, :], in0=s1[:w, :], scalar1=1.0 / D,
                            scalar2=None, op0=ALU.mult,
                        )
                        mu2 = pstats.tile([128, 1], F32, tag=f"mu2{bi}")
                        nc.vector.tensor_scalar(
                            out=mu2[:w, :], in0=mu[:w, :], scalar1=mu[:w, :],
                            scalar2=None, op0=ALU.mult,
                        )
                        var = pstats.tile([128, 1], F32, tag=f"var{bi}")
                        nc.vector.tensor_scalar(
                            out=var[:w, :], in0=ssq[:w, :], scalar1=1.0 / D,
                            scalar2=mu2[:w, :], op0=ALU.mult, op1=ALU.subtract,
                        )
                        std = pstats.tile([128, 1], F32, tag=f"std{bi}")
                        nc.scalar.activation(
                            std[:w, :], var[:w, :], ACTF.Sqrt, bias=eps_t[:w, :]
                        )
                        rstd = pstats.tile([128, 1], F32, tag=f"rstd{bi}")
                        nc.vector.reciprocal(rstd[:w, :], std[:w, :])
                        y = pwork.tile([128, 128], F16, tag=f"y{bi}")
                        nc.vector.tensor_scalar(
                            out=y[:w, :], in0=t_act[:w, :], scalar1=mu[:w, :],
                            scalar2=rstd[:w, :], op0=ALU.subtract, op1=ALU.mult,
                        )
                        ys.append(y)

                    out_t = pwork.tile([128, 128], F32, tag="out")
                    if affine:
                        a1 = pwork.tile([128, 128], F32, tag="a1")
                        nc.vector.tensor_tensor(a1[:w, :], ys[0][:w, :], g1bc[:w, :], ALU.mult)
                        a2 = pwork.tile([128, 128], F32, tag="a2")
                        nc.vector.tensor_tensor(a2[:w, :], ys[1][:w, :], g2bc[:w, :], ALU.mult)
                        nc.vector.tensor_tensor(a1[:w, :], a1[:w, :], a2[:w, :], ALU.add)
                        nc.vector.tensor_tensor(out_t[:w, :], a1[:w, :], bsbc[:w, :], ALU.add)
                    else:
                        nc.vector.tensor_tensor(out_t[:w, :], ys[0][:w, :], ys[1][:w, :], ALU.add)
                    nc.sync.dma_start(out_d[t * 128 : t * 128 + w, :], out_t[:w, :])

    nc.compile()
    return nc


def kernel(
    ego_embeddings,
    edge_row,
    edge_col,
    edge_val,
    W1,
    b1,
    W2,
    b2,
    gamma1,
    beta1,
    gamma2,
    beta2,
):
    ego = np.asarray(ego_embeddings, np.float32)
    W1 = np.asarray(W1, np.float32)
    W2 = np.asarray(W2, np.float32)
    b1 = np.asarray(b1, np.float32)
    b2 = np.asarray(b2, np.float32)
    gamma1 = np.asarray(gamma1, np.float32)
    gamma2 = np.asarray(gamma2, np.float32)
    beta1 = np.asarray(beta1, np.float32)
    beta2 = np.asarray(beta2, np.float32)

    affine = not (
        np.all(gamma1 == 1.0)
        and np.all(gamma2 == 1.0)
        and np.all(beta1 == 0.0)
        and np.all(beta2 == 0.0)
    )

    meta, per_core = _preprocess(ego, edge_row, edge_col, edge_val)
    nc = _build_program(meta, affine)

    table = np.ascontiguousarray(ego.astype(np.float16))
    in_maps = []
    for c in range(NCORES):
        r0 = c * ROWS_PER_CORE
        m = {
            "table": table,
            "egoT": np.ascontiguousarray(ego[r0 : r0 + ROWS_PER_CORE].T.astype(np.float16)),
            "idx": per_core[c]["idx"],
            "dl": per_core[c]["dl"],
            "val": per_core[c]["val"],
            "W1": W1.astype(np.float16),
            "W2": W2.astype(np.float16),
            "b1": b1.astype(np.float16).reshape(1, D),
            "b2": b2.astype(np.float16).reshape(1, D),
        }
        if affine:
            m["g1bc"] = np.tile(gamma1.reshape(1, D), (128, 1)).astype(np.float32)
            m["g2bc"] = np.tile(gamma2.reshape(1, D), (128, 1)).astype(np.float32)
            m["bsbc"] = np.tile((beta1 + beta2).reshape(1, D), (128, 1)).astype(np.float32)
        in_maps.append(m)

    res = run_bass_kernel_spmd(nc, in_maps, core_ids=list(range(NCORES)))
    return np.concatenate([res.results[c]["out"] for c in range(NCORES)], axis=0)


# revision 14
# speedup vs baseline: 2.9955x; 1.7170x over previous
"""GNN aggregator (KGAT-style bi-interaction) Trainium2 kernel.

side = segment_sum(edge_val * ego[edge_col], edge_row)       # SpMM, COO
out  = LN(leaky_relu((ego+side)@W1+b1)) + LN(leaky_relu((ego*side)@W2+b2))

Sharding: edges partitioned by destination row across 8 cores; core c owns
rows [c*12500, (c+1)*12500) and computes its slice of the output.

Per core, per destination tile of 128 rows:
  - source rows are fetched from a replicated fp16 copy of ego in HBM with
    dma_gather (int16 indices -> 4 chunks of 25000 rows; one SWDGE queue per
    chunk so descriptor generation runs on multiple Q7 cores in parallel)
  - segment-sum runs on the tensor engine: for each 128-edge block, a host-
    precomputed scatter matrix S [128e x 128dest] (S[e,d] = val_e if
    d == destloc_e else 0, streamed from HBM) is multiplied against the
    gathered messages, accumulating side^T[d, dest] in PSUM
  - the dense chain runs in fp16 with fp32 accumulation; LeakyReLU and the
    LN statistics ride the scalar engine (Lrelu / Square / Abs_reciprocal_sqrt
    with fused row-sum accumulators)
"""

import math

import numpy as np

import concourse.bacc as bacc
import concourse.bass as bass
import concourse.mybir as mybir
import concourse.tile as tile
from concourse import library_config
from concourse.bass_utils import run_bass_kernel_spmd

F16 = mybir.dt.float16
F32 = mybir.dt.float32
I16 = mybir.dt.int16
ALU = mybir.AluOpType
ACTF = mybir.ActivationFunctionType

N = 100000
D = 128
NCORES = 8
ROWS_PER_CORE = N // NCORES          # 12500
NTILES = math.ceil(ROWS_PER_CORE / 128)   # 98 (last tile 84 rows)
TILES_PER_ST = 7
NST = NTILES // TILES_PER_ST         # 14
CHUNK_ROWS = 25000                   # <= 32767 for int16 indices
NCHUNKS = math.ceil(N / CHUNK_ROWS)  # 4
LN_EPS = 1e-5
NEG_SLOPE = 0.01
DMA_SCRATCH = 16384
SINGLE_PACKET = False
NUM_SWDGE_QUEUES = 4
SIM_COMPAT = False   # True: avoid ACT funcs CoreSim lacks (Lrelu, Abs_reciprocal_sqrt)


def _tile_width(t):
    return min(128, ROWS_PER_CORE - t * 128)


def _preprocess(ego, edge_row, edge_col, edge_val):
    """Sort/shard edges on the host; build per-core gather indices and the
    scatter matrices S. B[t][c] (blocks per tile/chunk) is the max over
    cores so the SPMD program is shared."""
    edge_row = np.asarray(edge_row).astype(np.int64)
    edge_col = np.asarray(edge_col).astype(np.int64)
    edge_val = np.asarray(edge_val).astype(np.float32)

    core = edge_row // ROWS_PER_CORE
    local = edge_row - core * ROWS_PER_CORE
    tl = local // 128
    destloc = local - tl * 128
    ch = edge_col // CHUNK_ROWS
    idx16 = (edge_col - ch * CHUNK_ROWS).astype(np.int16)

    key = ((core * NTILES + tl) * NCHUNKS + ch).astype(np.int64)
    order = np.argsort(key, kind="stable")
    key_s = key[order]
    idx16_s = idx16[order]
    destloc_s = destloc[order]
    val_s = edge_val[order]

    counts = np.bincount(key_s, minlength=NCORES * NTILES * NCHUNKS).reshape(
        NCORES, NTILES, NCHUNKS
    )
    group_start = np.zeros(NCORES * NTILES * NCHUNKS + 1, np.int64)
    np.cumsum(counts.reshape(-1), out=group_start[1:])

    B = np.ceil(counts / 128).astype(np.int64).max(axis=0)  # [NTILES, NCHUNKS]
    empty = B.sum(axis=1) == 0
    B[empty, 0] = 1

    B_t = B.sum(axis=1)
    TOTB = int(B_t.sum())

    # per-(tile, chunk) gather calls: idx region of B[t,c]*8 columns each,
    # 16-row wrapped within the region; trailing pads are -1 and trimmed at
    # runtime via num_idxs_reg (per-core counts tensor)
    idx_col_off = np.zeros((NTILES, NCHUNKS), np.int64)
    acc = 0
    for t in range(NTILES):
        for c in range(NCHUNKS):
            idx_col_off[t, c] = acc
            acc += int(B[t, c]) * 8
    IDXCOLS = int(acc)

    tb_base = np.zeros(NTILES + 1, np.int64)
    np.cumsum(B_t, out=tb_base[1:])

    per_core = []
    for corei in range(NCORES):
        idx_all = np.full((16, IDXCOLS), -1, np.int16)
        cnt_all = np.zeros((1, NTILES * NCHUNKS), np.uint32)
        S_all = np.zeros((128, TOTB, 128), np.float16)
        for t in range(NTILES):
            jc = 0
            for c in range(NCHUNKS):
                g = (corei * NTILES + t) * NCHUNKS + c
                s0, s1 = group_start[g], group_start[g + 1]
                n = int(s1 - s0)
                nb = int(B[t, c])
                if nb == 0:
                    continue
                col0 = idx_col_off[t, c]
                if n == 0:
                    # keep one real index so the gather ucode never sees an
                    # all-negative list (S value is 0, so data is unused)
                    idx_all[0, col0] = 0
                    cnt_all[0, t * NCHUNKS + c] = 1
                else:
                    sl = np.arange(n)
                    idx_all[sl % 16, col0 + sl // 16] = idx16_s[s0:s1]
                    cnt_all[0, t * NCHUNKS + c] = n
                    jcols = tb_base[t] + jc + sl // 128
                    lanes = sl % 128
                    S_all[lanes, jcols, destloc_s[s0:s1]] = val_s[s0:s1]
                jc += nb
        per_core.append(
            {"idx": np.tile(idx_all, (8, 1)), "cnt": cnt_all, "S": S_all}
        )

    meta = dict(
        B=B, B_t=B_t, TOTB=TOTB,
        idx_col_off=idx_col_off, tb_base=tb_base, IDXCOLS=IDXCOLS,
    )
    return meta, per_core


def _build_program(meta, affine):
    B = meta["B"]
    B_t = meta["B_t"]
    TOTB = meta["TOTB"]
    idx_col_off = meta["idx_col_off"]
    tb_base = meta["tb_base"]
    IDXCOLS = meta["IDXCOLS"]

    nc = bacc.Bacc(
        "TRN2", target_bir_lowering=False, debug=False, num_devices=NCORES,
        dynamic_dma_scratch_size=DMA_SCRATCH, num_swdge_queues=NUM_SWDGE_QUEUES,
    )

    table = nc.dram_tensor("table", [N, D], F16, kind="ExternalInput")
    egoT = nc.dram_tensor("egoT", [D, ROWS_PER_CORE], F16, kind="ExternalInput")
    idx_d = nc.dram_tensor("idx", [128, IDXCOLS], I16, kind="ExternalInput")
    cnt_d = nc.dram_tensor("cnt", [1, NTILES * NCHUNKS], mybir.dt.uint32, kind="ExternalInput")
    S_d = nc.dram_tensor("S", [128, TOTB, 128], F16, kind="ExternalInput")
    W1_d = nc.dram_tensor("W1", [D, D], F16, kind="ExternalInput")
    W2_d = nc.dram_tensor("W2", [D, D], F16, kind="ExternalInput")
    b1_d = nc.dram_tensor("b1", [1, D], F16, kind="ExternalInput")
    b2_d = nc.dram_tensor("b2", [1, D], F16, kind="ExternalInput")
    if affine:
        g1_d = nc.dram_tensor("g1bc", [128, D], F32, kind="ExternalInput")
        g2_d = nc.dram_tensor("g2bc", [128, D], F32, kind="ExternalInput")
        bs_d = nc.dram_tensor("bsbc", [128, D], F32, kind="ExternalInput")
    out_d = nc.dram_tensor("out", [ROWS_PER_CORE, D], F32, kind="ExternalOutput")

    max_B_tc = int(B.max())
    max_TB_st = max(
        int(tb_base[(st + 1) * TILES_PER_ST] - tb_base[st * TILES_PER_ST])
        for st in range(NST)
    )
    STW = TILES_PER_ST * 128   # super-tile node width

    with tile.TileContext(nc) as tc:
        nc.gpsimd.load_library(library_config.mlp)
        with (
            tc.tile_pool(name="const", bufs=1) as pconst,
            tc.tile_pool(name="idx", bufs=2) as pidx,
            tc.tile_pool(name="msgs", bufs=2) as pmsgs,
            tc.tile_pool(name="S", bufs=2) as pS,
            tc.tile_pool(name="ego", bufs=2) as pego,
            tc.tile_pool(name="batch", bufs=2) as pbatch,
            tc.tile_pool(name="work", bufs=3) as pwork,
            tc.tile_pool(name="stats", bufs=8) as pstats,
            tc.tile_pool(name="psum", bufs=2, space="PSUM") as ppsum,
        ):
            W1 = pconst.tile([D, D], F16, tag="w1")
            nc.sync.dma_start(W1[:], W1_d[:])
            W2 = pconst.tile([D, D], F16, tag="w2")
            nc.sync.dma_start(W2[:], W2_d[:])
            b1 = pconst.tile([1, D], F16, tag="b1")
            nc.sync.dma_start(b1[:], b1_d[:])
            b2 = pconst.tile([1, D], F16, tag="b2")
            nc.sync.dma_start(b2[:], b2_d[:])
            ones = pconst.tile([1, 128], F16, tag="ones")
            nc.vector.memset(ones[:], 1.0)
            eps_t = pconst.tile([128, 1], F32, tag="eps")
            nc.vector.memset(eps_t[:], LN_EPS)
            if affine:
                g1bc = pconst.tile([128, D], F32, tag="g1")
                nc.sync.dma_start(g1bc[:], g1_d[:])
                g2bc = pconst.tile([128, D], F32, tag="g2")
                nc.sync.dma_start(g2bc[:], g2_d[:])
                bsbc = pconst.tile([128, D], F32, tag="bs")
                nc.sync.dma_start(bsbc[:], bs_d[:])

            cnt_sb = pconst.tile([1, NTILES * NCHUNKS], mybir.dt.uint32, tag="cnt")
            nc.sync.dma_start(cnt_sb[:], cnt_d[:])
            creg = ctx_reg = nc.gpsimd.new_register("gather_cnt")

            for st in range(NST):
                t0, t1 = st * TILES_PER_ST, (st + 1) * TILES_PER_ST
                # per-(tile, chunk) gather calls; idx DMA per (st, chunk)
                st_idx = {}
                for c in range(NCHUNKS):
                    co0 = int(idx_col_off[t0, c]) if B[t0, c] else None
                    # contiguous region for this st's chunk-c tiles
                    cols0 = int(idx_col_off[t0, c])
                    colsz = sum(int(B[t, c]) * 8 for t in range(t0, t1))
                    if colsz == 0:
                        continue
                    it = pidx.tile([128, max_B_tc * 8 * TILES_PER_ST], I16, tag=f"idx{c}")
                    nc.sync.dma_start(it[:, :colsz], idx_d[:, cols0 : cols0 + colsz])
                    st_idx[c] = (it, cols0)
                msgs = {}
                for t in range(t0, t1):
                    for c in range(NCHUNKS):
                        nb = int(B[t, c])
                        if nb == 0:
                            continue
                        it, cols0 = st_idx[c]
                        ioff = int(idx_col_off[t, c]) - cols0
                        ni = nb * 128
                        m = pmsgs.tile([128, max_B_tc, D], F16, tag=f"msgs{c}")
                        with tc.tile_critical():
                            nc.gpsimd.reg_load(
                                creg, cnt_sb[0:1, t * NCHUNKS + c : t * NCHUNKS + c + 1]
                            )
                            nc.gpsimd.dma_gather(
                                out_ap=m[:, :nb, :],
                                in_ap=table[c * CHUNK_ROWS : (c + 1) * CHUNK_ROWS, :],
                                idxs_ap=it[:, ioff : ioff + nb * 8],
                                num_idxs=ni,
                                num_idxs_reg=creg,
                                elem_size=D,
                                single_packet=(nb * 8 + 1 <= 64),
                                queue_num=c % NUM_SWDGE_QUEUES,
                            )
                        msgs[(t, c)] = m

                tb0 = int(tb_base[t0])
                tb_st = int(tb_base[t1]) - tb0
                S_t = pS.tile([128, max_TB_st, 128], F16, tag="S")
                nc.sync.dma_start(S_t[:, :tb_st, :], S_d[:, tb0 : tb0 + tb_st, :])

                stw = min(STW, ROWS_PER_CORE - t0 * 128)
                ego_b = pego.tile([128, STW], F16, tag="egoT")
                nc.sync.dma_start(ego_b[:, :stw], egoT[:, t0 * 128 : t0 * 128 + stw])
                side_b = pbatch.tile([128, STW], F16, tag="side")

                for t in range(t0, t1):
                    w = _tile_width(t)
                    bt = int(B_t[t])
                    ps = ppsum.tile([128, 128], F32, tag=f"side{(t - t0) % 2}")
                    j = 0
                    for c in range(NCHUNKS):
                        for b in range(int(B[t, c])):
                            col = int(tb_base[t]) - tb0 + j
                            nc.tensor.matmul(
                                ps[:, :w],
                                msgs[(t, c)][:, b, :],
                                S_t[:, col, :w],
                                start=(j == 0),
                                stop=(j == bt - 1),
                            )
                            j += 1
                    loc = (t - t0) * 128
                    nc.scalar.activation(
                        side_b[:, loc : loc + w], ps[:, :w], ACTF.Copy
                    )

                x1_b = pbatch.tile([128, STW], F16, tag="x1")
                nc.vector.tensor_tensor(x1_b[:, :stw], ego_b[:, :stw], side_b[:, :stw], ALU.add)
                x2_b = pbatch.tile([128, STW], F16, tag="x2")
                nc.vector.tensor_tensor(x2_b[:, :stw], ego_b[:, :stw], side_b[:, :stw], ALU.mult)

                for t in range(t0, t1):
                    w = _tile_width(t)
                    loc = (t - t0) * 128
                    ys = []
                    for bi, (xb, Wt, bt_) in enumerate(
                        ((x1_b, W1, b1), (x2_b, W2, b2))
                    ):
                        po = ppsum.tile([128, 128], F32, tag=f"o{bi}")
                        nc.tensor.matmul(
                            po[:w, :], xb[:, loc : loc + w], Wt[:],
                            start=True, stop=False,
                        )
                        nc.tensor.matmul(
                            po[:w, :], ones[:1, :w], bt_[:1, :],
                            start=False, stop=True,
                        )
                        # leaky_relu = relu(x) - slope*relu(-x); keeps all
                        # ACT funcs (Relu/Square/Abs_reciprocal_sqrt/Copy) in
                        # one activation-table set -> no mid-kernel reloads
                        r1 = pwork.tile([128, 128], F16, tag=f"r1{bi}")
                        sp = pstats.tile([128, 1], F32, tag=f"sp{bi}")
                        nc.scalar.activation(
                            r1[:w, :], po[:w, :], ACTF.Relu, accum_out=sp[:w, :]
                        )
                        r2 = pwork.tile([128, 128], F16, tag=f"r2{bi}")
                        sn = pstats.tile([128, 1], F32, tag=f"sn{bi}")
                        nc.scalar.activation(
                            r2[:w, :], po[:w, :], ACTF.Relu, scale=-1.0,
                            accum_out=sn[:w, :],
                        )
                        t_act = pwork.tile([128, 128], F16, tag=f"t{bi}")
                        nc.vector.scalar_tensor_tensor(
                            out=t_act[:w, :], in0=r2[:w, :], scalar=-NEG_SLOPE,
                            in1=r1[:w, :], op0=ALU.mult, op1=ALU.add,
                        )
                        sq = pwork.tile([128, 128], F16, tag=f"sq{bi}")
                        ssq = pstats.tile([128, 1], F32, tag=f"ssq{bi}")
                        # scale=1/sqrt(D): accumulator yields sum(t^2)/D
                        nc.scalar.activation(
                            sq[:w, :], t_act[:w, :], ACTF.Square,
                            scale=float(1.0 / math.sqrt(D)), accum_out=ssq[:w, :],
                        )
                        s1 = pstats.tile([128, 1], F32, tag=f"sum{bi}")
                        nc.vector.tensor_scalar(
                            out=s1[:w, :], in0=sn[:w, :], scalar1=-NEG_SLOPE,
                            scalar2=sp[:w, :], op0=ALU.mult, op1=ALU.add,
                        )
                        mu = pstats.tile([128, 1], F32, tag=f"mu{bi}")
                        nc.vector.tensor_scalar(
                            out=mu[:w, :], in0=s1[:w, :], scalar1=1.0 / D,
                            scalar2=None, op0=ALU.mult,
                        )
                        rstd = pstats.tile([128, 1], F32, tag=f"rstd{bi}")
                        if SIM_COMPAT:
                            var = pstats.tile([128, 1], F32, tag=f"var{bi}")
                            nc.vector.scalar_tensor_tensor(
                                out=var[:w, :], in0=mu[:w, :], scalar=mu[:w, :],
                                in1=ssq[:w, :], op0=ALU.mult, op1=ALU.subtract,
                            )
                            std = pstats.tile([128, 1], F32, tag=f"std{bi}")
                            nc.scalar.activation(
                                std[:w, :], var[:w, :], ACTF.Sqrt,
                                scale=-1.0, bias=eps_t[:w, :],
                            )
                            nc.vector.reciprocal(rstd[:w, :], std[:w, :])
                        else:
                            negvar = pstats.tile([128, 1], F32, tag=f"nv{bi}")
                            nc.vector.scalar_tensor_tensor(
                                out=negvar[:w, :], in0=mu[:w, :], scalar=mu[:w, :],
                                in1=ssq[:w, :], op0=ALU.mult, op1=ALU.subtract,
                            )
                            nc.scalar.activation(
                                rstd[:w, :], negvar[:w, :],
                                ACTF.Abs_reciprocal_sqrt,
                                scale=-1.0, bias=eps_t[:w, :],
                            )
                        y = pwork.tile([128, 128], F16, tag=f"y{bi}")
                        nc.vector.tensor_scalar(
                            out=y[:w, :], in0=t_act[:w, :], scalar1=mu[:w, :],
                            scalar2=rstd[:w, :], op0=ALU.subtract, op1=ALU.mult,
                        )
                        ys.append(y)

                    out_t = pwork.tile([128, 128], F32, tag="out")
                    if affine:
                        a1 = pwork.tile([128, 128], F32, tag="a1")
                        nc.vector.tensor_tensor(a1[:w, :], ys[0][:w, :], g1bc[:w, :], ALU.mult)
                        a2 = pwork.tile([128, 128], F32, tag="a2")
                        nc.vector.tensor_tensor(a2[:w, :], ys[1][:w, :], g2bc[:w, :], ALU.mult)
                        nc.vector.tensor_tensor(a1[:w, :], a1[:w, :], a2[:w, :], ALU.add)
                        nc.vector.tensor_tensor(out_t[:w, :], a1[:w, :], bsbc[:w, :], ALU.add)
                    else:
                        nc.vector.tensor_tensor(
                            out_t[:w, :], ys[0][:w, :], ys[1][:w, :], ALU.add
                        )
                    nc.sync.dma_start(out_d[t * 128 : t * 128 + w, :], out_t[:w, :])

    nc.compile()
    return nc


def _make_in_maps(ego, W1, b1, W2, b2, gamma1, beta1, gamma2, beta2, per_core, affine):
    table = np.ascontiguousarray(ego.astype(np.float16))
    in_maps = []
    for c in range(NCORES):
        r0 = c * ROWS_PER_CORE
        m = {
            "table": table,
            "egoT": np.ascontiguousarray(ego[r0 : r0 + ROWS_PER_CORE].T.astype(np.float16)),
            "idx": per_core[c]["idx"],
            "cnt": per_core[c]["cnt"],
            "S": per_core[c]["S"],
            "W1": W1.astype(np.float16),
            "W2": W2.astype(np.float16),
            "b1": b1.astype(np.float16).reshape(1, D),
            "b2": b2.astype(np.float16).reshape(1, D),
        }
        if affine:
            m["g1bc"] = np.tile(gamma1.reshape(1, D), (128, 1)).astype(np.float32)
            m["g2bc"] = np.tile(gamma2.reshape(1, D), (128, 1)).astype(np.float32)
            m["bsbc"] = np.tile((beta1 + beta2).reshape(1, D), (128, 1)).astype(np.float32)
        in_maps.append(m)
    return in_maps


def kernel(
    ego_embeddings, edge_row, edge_col, edge_val,
    W1, b1, W2, b2, gamma1, beta1, gamma2, beta2,
):
    ego = np.asarray(ego_embeddings, np.float32)
    W1 = np.asarray(W1, np.float32)
    W2 = np.asarray(W2, np.float32)
    b1 = np.asarray(b1, np.float32)
    b2 = np.asarray(b2, np.float32)
    gamma1 = np.asarray(gamma1, np.float32)
    gamma2 = np.asarray(gamma2, np.float32)
    beta1 = np.asarray(beta1, np.float32)
    beta2 = np.asarray(beta2, np.float32)

    affine = not (
        np.all(gamma1 == 1.0) and np.all(gamma2 == 1.0)
        and np.all(beta1 == 0.0) and np.all(beta2 == 0.0)
    )

    meta, per_core = _preprocess(ego, edge_row, edge_col, edge_val)
    nc = _build_program(meta, affine)
    in_maps = _make_in_maps(
        ego, W1, b1, W2, b2, gamma1, beta1, gamma2, beta2, per_core, affine
    )
    res = run_bass_kernel_spmd(nc, in_maps, core_ids=list(range(NCORES)))
    return np.concatenate([res.results[c]["out"] for c in range(NCORES)], axis=0)
